# revision 12
# baseline (speedup 1.0000x reference)
"""Trainium2 Bass kernel for nn_AdditiveAttention (B=32, NQ=1, NK=4096, D=512, H=256).

Data-parallel over 8 NeuronCores: each core owns 4 batches. Per core:
  kprojT[h, t] = sum_d W_k[d, h] * keys[b, t, d]      (PE, fp16, W_k stationary)
  featT        = tanh(kprojT + qproj_b)               (ACT, bias fused, fp16 out)
  scores[t]    = sum_h w_v[h] * featT[h, t]           (PE matvec, fp16)
  out[b, t]    = softmax_t(scores) * values[b, t]     (exp straight from PSUM with
                                                       fused partial sums; scores
                                                       are O(4) so no max-subtract)

The keys shard is handed to the device pre-transposed ([4, 512, 4096]) and
pre-cast to fp16 (the kernel's compute precision) so the contraction dim lands
on SBUF partitions. Each batch's keys load is a single 3D-access-pattern DMA
(the ~0.6us per-DMA issue cost on the sync sequencer serializes, so fewer,
bigger DMAs win). A few self-matmuls on W_q at the start warm the PE HAM
clock-gate to 2.4 GHz before the real matmul stream begins.
"""

import numpy as np

N_CORES = 8
B, NQ, NK, D, H = 32, 1, 4096, 512, 256
B_LOC = B // N_CORES  # 4 batches per core
KT = D // 128         # 4 contraction tiles
HT = H // 128         # 2 hidden tiles
TOKC = 512            # matvec chunk (= one PSUM bank of f32)
TOKP = 1024           # kproj/tanh chunk (2 PSUM banks)
NCP = NK // TOKP      # 4 kproj chunks per batch
QTOK = NK // 4        # batch-0 quarter width (ramp)
N_WARM = 8            # HAM warmup matmuls


def _install_profile_hook():
    """Make trace=True / BASS_TRACE=1 usable when the image's antenv lacks
    axon_hooks (degrades silently if anything is missing)."""
    try:
        from antenv import axon_hooks  # noqa: F401
        return
    except ImportError:
        pass
    try:
        import sys
        import types

        import antenv
        from trn_agent_boot.trn_boot import _ntff_profile_via_ctypes

        mod = types.ModuleType("antenv.axon_hooks")
        mod._h = None
        mod.set_axon_ntff_profile_hook = lambda h: setattr(mod, "_h", h)
        mod.get_axon_ntff_profile_hook = lambda: mod._h
        antenv.axon_hooks = mod
        sys.modules["antenv.axon_hooks"] = mod
        mod._h = _ntff_profile_via_ctypes("/opt/axon/libaxon_pjrt.so")
    except Exception:
        pass


def build_nc():
    import concourse.tile as tile
    from concourse import bacc, mybir

    f32 = mybir.dt.float32
    f16 = mybir.dt.float16
    Act = mybir.ActivationFunctionType
    AX = mybir.AxisListType.X

    nc = bacc.Bacc("TRN2", target_bir_lowering=False, debug=False,
                   num_devices=N_CORES)

    keysT_ext = nc.dram_tensor("keysT", [B_LOC, D, NK], f16, kind="ExternalInput")
    qT_ext = nc.dram_tensor("queriesT", [128, KT * B_LOC], f32, kind="ExternalInput")
    vals_ext = nc.dram_tensor("vals", [B_LOC, NK], f32, kind="ExternalInput")
    wk_ext = nc.dram_tensor("wk", [128, KT * H], f16, kind="ExternalInput")
    wq_ext = nc.dram_tensor("wq", [128, KT * H], f32, kind="ExternalInput")
    wv_ext = nc.dram_tensor("wv", [128, HT], f16, kind="ExternalInput")
    out_ext = nc.dram_tensor("out", [B_LOC, NK], f32, kind="ExternalOutput")

    # [B_LOC, D, NK] viewed so one DMA can pull [128 part, KT, ntok]
    keys3d = keysT_ext.ap().rearrange("b (k p) n -> b k p n", p=128)

    with tile.TileContext(nc) as tc:
        with (
            tc.tile_pool(name="keys", bufs=3) as keys_pool,
            tc.tile_pool(name="keys0", bufs=4) as keys0_pool,
            tc.tile_pool(name="feat", bufs=6) as feat_pool,
            tc.tile_pool(name="static", bufs=1) as st,
            tc.tile_pool(name="kp", bufs=3, space="PSUM") as kp_pool,
            tc.tile_pool(name="sc", bufs=2, space="PSUM") as sc_pool,
        ):
            # ---- HAM warmup on memset data: PE activity needs no DMA, so
            # the clock-gate reaches 8/8 before the first real matmul ----
            wtile = st.tile([128, H], f32, tag="warm_in")
            nc.vector.memset(wtile[:], 1.0)
            warm_ps = sc_pool.tile([128, H], f32, tag="sc")
            for w in range(N_WARM):
                nc.tensor.matmul(warm_ps[:], wtile[:, 0:128], wtile[:],
                                 start=(w == 0), stop=(w == N_WARM - 1))
            warm_out = st.tile([128, 1], f32, tag="warm")
            nc.vector.reduce_max(warm_out[:], warm_ps[:], axis=AX)

            # ---- loads: W_k and batch-0 keys first (gate the first real
            # matmuls), then the q-side, then the rest of the keys ----
            wk_sb = st.tile([128, KT, H], f16, tag="wk")
            nc.sync.dma_start(wk_sb[:], wk_ext.ap())
            kt_tiles = {}
            for q in range(4):
                t = keys0_pool.tile([128, KT, QTOK], f16, tag="kt0")
                nc.sync.dma_start(t[:], keys3d[0, :, :, q * QTOK:(q + 1) * QTOK]
                                  .rearrange("k p n -> p k n"))
                kt_tiles[(0, q)] = t
            wq_sb = st.tile([128, KT, H], f32, tag="wq")
            nc.sync.dma_start(wq_sb[:], wq_ext.ap())
            qin_sb = st.tile([128, KT, B_LOC], f32, tag="qin")
            nc.sync.dma_start(qin_sb[:], qT_ext.ap())
            wv_sb = st.tile([128, HT], f16, tag="wv")
            nc.sync.dma_start(wv_sb[:], wv_ext.ap())
            # per-batch softmax rows live at partition 32*b (engine ops need
            # 32-aligned base partitions); vals/out ride the scalar HWDGE
            # queue so they never sit behind the big keys DMAs
            vals_sb = st.tile([128, NK], f32, tag="vals")
            for b in range(B_LOC):
                nc.scalar.dma_start(vals_sb[32 * b:32 * b + 1, :],
                                    vals_ext[b:b + 1, :])
            for b in range(1, B_LOC):
                t = keys_pool.tile([128, KT, NK], f16, tag="kt")
                nc.sync.dma_start(t[:], keys3d[b].rearrange("k p n -> p k n"))
                kt_tiles[b] = t

            # ---- qproj (f32, exact): qbias[h][:, b] = (queries @ W_q)^T ----
            qbias_sb = st.tile([128, HT, B_LOC], f32, tag="qbias")
            for h in range(HT):
                qp = sc_pool.tile([128, B_LOC], f32, tag="sc")
                for k in range(KT):
                    nc.tensor.matmul(
                        qp[:],
                        wq_sb[:, k, h * 128:(h + 1) * 128],
                        qin_sb[:, k, :],
                        start=(k == 0), stop=(k == KT - 1),
                    )
                nc.vector.tensor_copy(qbias_sb[:, h, :], qp[:])

            # ---- per-batch softmax state (row 32*b per batch) ----
            esc_sb = st.tile([128, NK], f32, tag="esc")       # exp(scores)*vals
            psum_sb = st.tile([128, NK // TOKC], f32, tag="psums")
            ssum_sb = st.tile([128, 1], f32, tag="ssum")
            recip_sb = st.tile([128, 1], f32, tag="recip")

            for b in range(B_LOC):
                r = 32 * b

                def ksrc(c0):
                    """keys AP maker for tokens starting at c0 of this batch."""
                    if b == 0:
                        q = c0 // QTOK
                        t = kt_tiles[(0, q)]
                        o = c0 - q * QTOK
                        return lambda k, j: t[:, k, o + j * TOKC:o + (j + 1) * TOKC]
                    t = kt_tiles[b]
                    return lambda k, j: t[:, k, c0 + j * TOKC:c0 + (j + 1) * TOKC]

                for cp in range(NCP):
                    c0 = cp * TOKP
                    src = ksrc(c0)
                    fts = []
                    for h in range(HT):
                        ps = kp_pool.tile([128, TOKP], f32, tag="ps")
                        for j in range(2):
                            for k in range(KT):
                                nc.tensor.matmul(
                                    ps[:, j * TOKC:(j + 1) * TOKC],
                                    wk_sb[:, k, h * 128:(h + 1) * 128],
                                    src(k, j),
                                    start=(k == 0), stop=(k == KT - 1),
                                )
                        ft = feat_pool.tile([128, TOKP], f16, tag="ft")
                        nc.scalar.activation(ft[:], ps[:], Act.Tanh,
                                             bias=qbias_sb[:, h, b:b + 1])
                        fts.append(ft)
                    for j in range(2):
                        sc = sc_pool.tile([128, TOKC], f32, tag="sc")
                        for h in range(HT):
                            nc.tensor.matmul(
                                sc[r:r + 1, :], wv_sb[:, h:h + 1],
                                fts[h][:, j * TOKC:(j + 1) * TOKC],
                                start=(h == 0), stop=(h == HT - 1),
                                tile_position=(0, r))
                        c = 2 * cp + j
                        cs = c * TOKC
                        nc.scalar.activation(esc_sb[r:r + 1, cs:cs + TOKC],
                                             sc[r:r + 1, :], Act.Exp,
                                             accum_out=psum_sb[r:r + 1, c:c + 1])
                        nc.vector.tensor_mul(esc_sb[r:r + 1, cs:cs + TOKC],
                                             esc_sb[r:r + 1, cs:cs + TOKC],
                                             vals_sb[r:r + 1, cs:cs + TOKC])

                # softmax denominator; scale in one tensor_scalar at batch end
                nc.vector.reduce_sum(ssum_sb[r:r + 1, :], psum_sb[r:r + 1, :],
                                     axis=AX)
                nc.vector.reciprocal(recip_sb[r:r + 1, :], ssum_sb[r:r + 1, :])
                for g in range(2):
                    gs = g * (NK // 2)
                    nc.vector.tensor_scalar_mul(
                        esc_sb[r:r + 1, gs:gs + NK // 2],
                        esc_sb[r:r + 1, gs:gs + NK // 2],
                        recip_sb[r:r + 1, :])
                    nc.scalar.dma_start(out_ext[b:b + 1, gs:gs + NK // 2],
                                        esc_sb[r:r + 1, gs:gs + NK // 2])

    nc.compile()
    return nc


def shard_inputs(queries, keys, values, W_q, W_k, w_v):
    queries = np.asarray(queries, np.float32)
    keys = np.asarray(keys, np.float32)
    values = np.asarray(values, np.float32)
    W_q = np.asarray(W_q, np.float32)
    W_k = np.asarray(W_k, np.float32)
    w_v = np.asarray(w_v, np.float32)

    def merge_kt(w, ncol):  # [KT*128, ncol] -> [128, KT*ncol] partition-major
        return np.ascontiguousarray(
            w.reshape(KT, 128, ncol).transpose(1, 0, 2).reshape(128, KT * ncol))

    wk2 = merge_kt(W_k, H).astype(np.float16)
    wq2 = merge_kt(W_q, H)
    wv2 = np.ascontiguousarray(w_v.reshape(HT, 128).T).astype(np.float16)
    in_maps = []
    for i in range(N_CORES):
        b0, b1 = i * B_LOC, (i + 1) * B_LOC
        qT = np.ascontiguousarray(queries[b0:b1, 0, :].T)  # [512, B_LOC]
        in_maps.append({
            "keysT": np.ascontiguousarray(
                keys[b0:b1].transpose(0, 2, 1)).astype(np.float16),
            "queriesT": merge_kt(qT, B_LOC),
            "vals": np.ascontiguousarray(values[b0:b1, :, 0]),
            "wk": wk2, "wq": wq2, "wv": wv2,
        })
    return in_maps


_NC_CACHE = {}


def run(in_maps, trace=False, tmpdir=None):
    from concourse.bass_utils import run_bass_kernel_spmd

    _install_profile_hook()
    if trace:
        # no artifact bucket inside the container; keep traces local
        import concourse.bass_utils as bu
        bu.upload_artifacts = lambda d: "local://" + d
    if "nc" not in _NC_CACHE:
        _NC_CACHE["nc"] = build_nc()
    nc = _NC_CACHE["nc"]
    return run_bass_kernel_spmd(nc, in_maps, core_ids=list(range(N_CORES)),
                                trace=trace, tmpdir=tmpdir)


def kernel(queries, keys, values, W_q, W_k, w_v):
    in_maps = shard_inputs(queries, keys, values, W_q, W_k, w_v)
    res = run(in_maps)
    return np.concatenate([res.results[i]["out"] for i in range(N_CORES)], axis=0)
